# revision 11
# baseline (speedup 1.0000x reference)
"""FFM layer kernel for 8 Trainium2 NeuronCores (fp8 DoubleRow edition, v3).

Math (reference): x[B,39] = 13 dense cols + 26 sparse index cols (ints 0..99
stored as f32).  inputs[B,2613] = [dense | one_hot(sparse)], then
  linear = inputs @ w.T + b
  field  = einsum('bn,nfk->bfk', inputs, v)        # [B,39,16]
  cross  = 0.5*sum_k((sum_f field)^2 - sum_f field^2)
  out    = sigmoid(linear + cross)

Strategy: data-parallel over batch, 2048 rows/core.  The one-hot matrix is
built host-side directly in fp8 (it is exactly the same number of bytes as
the index tensor an on-device build would need, so DMA cost is unchanged and
the DVE/GPSIMD build cost disappears entirely) and used as the stationary
operand of fp8 DoubleRow matmuls (256-row contraction per instruction, 2x
the fp16 PE rate):
  psum[128b, 658] = sum_pairs ohp[128f,2,128b].T @ vp[128f,2,658]
Columns: 624 field cols (k-major), then 16+16 split-precision s columns
(s_hi = fp8(A*512), s_res = fp8(residual*16)/16 with A = sum_f v — two fp8
levels give s = sum_f field to ~0.4% of |A|, so no row-reduce over fields is
needed), then w_hi/w_res (same trick for the linear term).  Dense x
precision is recovered by 13 residual feature rows carrying 16*(x16-x8)
against v/16, reusing rows that were zero padding.  Feature rows: [1s row
(bias) | 13 dense x | 13 dense residual | pad to 32 | 26*100 one-hot | zero
tail], packed into 22 chunks of 128 = 11 DoubleRow pairs (chunk 21 zero).
v scaled by 2048 into e4m3's normal range, fp32 PSUM accumulation.

Epilogue: ACT does the two Square-accums (sum field^2) and a per-group
batched sigmoid (logit pre-combined so the bias is 0); DVE does the s/w
column-pair reduces and the [p,4] logit combines.  Throwaway warmup matmuls
release the HAM clock throttle during the DMA head; host tensors are
partition-major and group-blocked so DMAs move long contiguous runs and the
first group's one-hot lands early.
"""

import sys

sys.path.insert(0, "/opt/trn_rl_repo")

import numpy as np
import ml_dtypes

import concourse.tile as tile
from concourse import bacc, mybir
from concourse.bass_utils import run_bass_kernel_spmd

N_CORES = 8
B_FULL = 16384
BC = B_FULL // N_CORES  # 2048 rows per core
P = 128
N_DENSE = 13
N_SPARSE = 26
SPARSE_DIM = 100
N_FIELD = 39
K_DIM = 16
NCHUNK = 21             # chunks carrying real feature rows
NCTOT = 22              # +1 zero chunk so DoubleRow pairs cover everything
NPAIR = NCTOT // 2
RTOT = NCHUNK * P       # 2688 real feature rows
# device feature rows: 0 = const-ones (bias), 1..13 = dense x, 14..26 =
# dense residual 16*(x16-x8), 27..31 = zero, then 26*100 one-hot, zero tail
SP0 = 32                # first one-hot row
NFEAT_END = SP0 + N_SPARSE * SPARSE_DIM  # 2632
NFCOL = N_FIELD * K_DIM  # 624 field cols
COLS0 = 512             # ps0: field cols 0:512
COLS = NFCOL + 2 * K_DIM + 2  # 658
COLS1 = COLS - COLS0    # 146: field 112 + s_hi 16 + s_res 16 + whi + wres
SH0 = NFCOL - COLS0     # 112 (ps1-local start of s_hi)
WH0 = SH0 + 2 * K_DIM   # 144 (ps1-local start of w cols)
GB = 4                  # batch tiles per group

SCALE_V = 2048.0
SCALE_W = 8192.0
SCALE_S = 512.0
RES = 16.0              # dense residual row scale-up

F8 = mybir.dt.float8e4
F32 = mybir.dt.float32
F16 = mybir.dt.float16
NP_F8 = ml_dtypes.float8_e4m3

_prog_cache = {}


def _build_program(bc):
    """One SPMD program for a batch slice of `bc` rows (all cores identical)."""
    nbt = bc // P
    ngroups = nbt // GB
    assert nbt % GB == 0
    gw = GB * P

    nc = bacc.Bacc("TRN2", target_bir_lowering=False, debug=False)
    # oh is group-blocked so each group's slice lands in one burst of long
    # contiguous per-partition runs; vp is partition-major for the same
    # reason
    oh_d = nc.declare_dram_parameter(
        "oh", [ngroups, P, NCTOT, gw], F8, isOutput=False)
    vp_d = nc.declare_dram_parameter("vp", [P, NCTOT, COLS], F8, isOutput=False)
    y_d = nc.declare_dram_parameter("y", [P, nbt], F32, isOutput=True)

    # per-group oh sub-DMAs over pair ranges, split across the sync and
    # gpsimd HWDGE queues (vp owns the scalar queue so it is never stuck
    # behind a one-hot burst — it gates every matmul); small first sub so
    # pair 0 lands early and the first matmuls start
    OSUB = [(0, 1), (1, 3), (3, 6), (6, 9), (9, NPAIR)]
    OSUB_ENG = ("sync", "gpsimd", "sync", "gpsimd", "sync")
    VSUB = [(0, 2), (2, 12), (12, NCTOT)]
    VSUB_ENG = ("scalar", "scalar", "scalar")

    with tile.TileContext(nc) as tc:
        with (
            tc.tile_pool(name="pers", bufs=1) as pers,
            tc.tile_pool(name="psum", bufs=3, space="PSUM") as psum,
            tc.tile_pool(name="epi", bufs=3) as epi,
            tc.tile_pool(name="grp", bufs=2) as grp,
        ):
            oh_p = []
            for j in range(NPAIR):
                oh_p.append(pers.tile([P, 2, bc], F8, tag=f"ohp{j}",
                                      name=f"ohp{j}"))
            y_all = pers.tile([P, nbt], F32, tag="yall")
            vp_all = pers.tile([P, NCTOT, COLS], F8, tag="vp")

            def load_oh(g):
                c0 = g * gw
                for (lo, hi), ename in zip(OSUB, OSUB_ENG):
                    eng = getattr(nc, ename)
                    for j in range(lo, hi):
                        eng.dma_start(oh_p[j][:, :, c0:c0 + gw],
                                      oh_d[g, :, 2 * j:2 * j + 2, :])

            def load_vp(lo, hi, ename):
                getattr(nc, ename).dma_start(
                    vp_all[:, lo:hi, :], vp_d[:, lo:hi, :])

            # warmup scratch first so the memsets aren't stuck behind DMA
            # issue on the gpsimd queue
            wz16 = pers.tile([P, 16], F16, tag="wz16")
            wz512 = pers.tile([P, 512], F16, tag="wz512")
            nc.gpsimd.memset(wz16[:], 0.0)
            nc.gpsimd.memset(wz512[:], 0.0)

            # vp chunks 0-1 first: they gate the first matmuls
            load_vp(*VSUB[0], VSUB_ENG[0])
            load_oh(0)
            for (lo, hi), ename in list(zip(VSUB, VSUB_ENG))[1:]:
                load_vp(lo, hi, ename)

            # PE warmup: throwaway matmuls on zeroed tiles during the DMA
            # head release the HAM clock throttle (cold PE runs slow until
            # ~3.4us of sustained activity); sized to end right when the
            # first one-hot pair and vp chunks have landed
            wps = psum.tile([P, 512], F32, tag="warmps", name="warmps", bufs=1)
            for _ in range(6):
                nc.tensor.matmul(wps[0:16, 0:512], wz16[:], wz512[:],
                                 start=True, stop=True)
            for _ in range(6):
                nc.tensor.matmul(wps[0:16, 0:64], wz16[:], wz512[:, 0:64],
                                 start=True, stop=True)

            for g in range(ngroups):
                if g + 1 < ngroups:
                    load_oh(g + 1)
                # per-group accumulator strips: one [128, GB] f32 tile each
                sq0b = grp.tile([P, GB], F32, tag="sq0b")
                sq1b = grp.tile([P, GB], F32, tag="sq1b")
                s2b = grp.tile([P, GB], F32, tag="s2b")
                ub = grp.tile([P, GB], F32, tag="ub")
                for b4 in range(GB):
                    bt = g * GB + b4
                    bs = slice(bt * P, (bt + 1) * P)
                    ps0 = psum.tile([P, COLS0], F32, tag="ps0")
                    ps1 = psum.tile([P, COLS1], F32, tag="ps1")
                    for j in range(NPAIR):
                        lhs = oh_p[j][:, :, bs]
                        nc.tensor.matmul(
                            ps0[:], lhs, vp_all[:, 2 * j:2 * j + 2, 0:COLS0],
                            start=(j == 0), stop=(j == NPAIR - 1),
                            perf_mode=mybir.MatmulPerfMode.DoubleRow,
                        )
                        nc.tensor.matmul(
                            ps1[:], lhs, vp_all[:, 2 * j:2 * j + 2, COLS0:COLS],
                            start=(j == 0), stop=(j == NPAIR - 1),
                            perf_mode=mybir.MatmulPerfMode.DoubleRow,
                        )
                    # s = s_hi + s_res (sigma=512 units); u = lin*8192
                    s_t = epi.tile([P, K_DIM], F32, tag="s")
                    nc.vector.tensor_reduce(
                        out=s_t[:],
                        in_=ps1[:, SH0:SH0 + 2 * K_DIM].rearrange(
                            "p (two k) -> p k two", two=2),
                        axis=mybir.AxisListType.X,
                        op=mybir.AluOpType.add,
                    )
                    nc.vector.tensor_reduce(
                        out=ub[:, b4:b4 + 1], in_=ps1[:, WH0:WH0 + 2],
                        axis=mybir.AxisListType.X,
                        op=mybir.AluOpType.add,
                    )
                    # sum field^2 on ACT: 512 cols from ps0, 112 from ps1
                    sq_scr = epi.tile([P, COLS0], F32, tag="sqscr")
                    nc.scalar.activation(
                        out=sq_scr[:], in_=ps0[:],
                        func=mybir.ActivationFunctionType.Square,
                        scale=1.0 / SCALE_V,
                        accum_out=sq0b[:, b4:b4 + 1],
                    )
                    sq_scr1 = epi.tile([P, SH0], F32, tag="sqscr1")
                    nc.scalar.activation(
                        out=sq_scr1[:], in_=ps1[:, 0:SH0],
                        func=mybir.ActivationFunctionType.Square,
                        scale=1.0 / SCALE_V,
                        accum_out=sq1b[:, b4:b4 + 1],
                    )
                    s2_scr = epi.tile([P, K_DIM], F32, tag="s2scr")
                    nc.scalar.activation(
                        out=s2_scr[:], in_=s_t[:],
                        func=mybir.ActivationFunctionType.Square,
                        scale=1.0 / SCALE_S,
                        accum_out=s2b[:, b4:b4 + 1],
                    )
                # batched logit combine + sigmoid for the group's 4 tiles:
                # L2 = (s2 - sq) + lin*2, y = Sigmoid(0.5*L2)
                sqs = grp.tile([P, GB], F32, tag="sqs")
                nc.vector.scalar_tensor_tensor(
                    out=sqs[:], in0=sq0b[:], scalar=1.0, in1=sq1b[:],
                    op0=mybir.AluOpType.mult, op1=mybir.AluOpType.add)
                dl = grp.tile([P, GB], F32, tag="dl")
                nc.vector.scalar_tensor_tensor(
                    out=dl[:], in0=sqs[:], scalar=-1.0, in1=s2b[:],
                    op0=mybir.AluOpType.mult, op1=mybir.AluOpType.add)
                l2 = grp.tile([P, GB], F32, tag="l2")
                nc.vector.scalar_tensor_tensor(
                    out=l2[:], in0=ub[:], scalar=2.0 / SCALE_W, in1=dl[:],
                    op0=mybir.AluOpType.mult, op1=mybir.AluOpType.add)
                nc.scalar.activation(
                    out=y_all[:, g * GB:(g + 1) * GB], in_=l2[:],
                    func=mybir.ActivationFunctionType.Sigmoid,
                    scale=0.5,
                )
                nc.scalar.dma_start(y_d[:, g * GB:(g + 1) * GB],
                                    y_all[:, g * GB:(g + 1) * GB])

    nc.compile()
    return nc


def _get_program(bc):
    if bc not in _prog_cache:
        _prog_cache[bc] = _build_program(bc)
    return _prog_cache[bc]


def _q8(a):
    return np.asarray(a, np.float32).astype(NP_F8).astype(np.float32)


def _prep_shared(w_weight, w_bias, v):
    """vp[P, 22, 658] fp8 (same on every core)."""
    w = w_weight[0].astype(np.float64)
    v_km = np.ascontiguousarray(
        v.astype(np.float64).transpose(0, 2, 1)).reshape(2613, NFCOL)
    A = v.astype(np.float64).sum(axis=1)  # [2613, 16]

    nrows = NCTOT * P
    vp = np.zeros((nrows, COLS), np.float32)

    def fill(rows, vblk, ablk, wblk):
        """Two-level fp8 quantization of a (v, A, w) block into device rows."""
        vp[rows, 0:NFCOL] = _q8(vblk * SCALE_V)
        s_hi = _q8(ablk * SCALE_S)
        vp[rows, NFCOL:NFCOL + K_DIM] = s_hi
        vp[rows, NFCOL + K_DIM:NFCOL + 2 * K_DIM] = _q8(
            (ablk - s_hi / SCALE_S) * SCALE_S * 16.0) / 16.0
        w_hi = _q8(wblk * SCALE_W)
        vp[rows, COLS - 2] = w_hi
        vp[rows, COLS - 1] = _q8((wblk - w_hi / SCALE_W) * SCALE_W * 16.0) / 16.0

    dn = np.arange(1, 1 + N_DENSE)
    fill(dn, v_km[:N_DENSE], A[:N_DENSE], w[:N_DENSE])
    dr = np.arange(14, 14 + N_DENSE)
    fill(dr, v_km[:N_DENSE] / RES, A[:N_DENSE] / RES, w[:N_DENSE] / RES)
    sp = np.arange(SP0, NFEAT_END)
    fill(sp, v_km[N_DENSE:], A[N_DENSE:], w[N_DENSE:])
    # bias via the const-ones row's w columns
    wb = float(w_bias[0])
    wb_hi = _q8(wb * SCALE_W)
    vp[0, COLS - 2] = wb_hi
    vp[0, COLS - 1] = _q8((wb - wb_hi / SCALE_W) * SCALE_W * 16.0) / 16.0

    vp8 = np.ascontiguousarray(
        vp.astype(NP_F8).reshape(NCTOT, P, COLS).transpose(1, 0, 2))
    return vp8


def _prep_core(x_core):
    """Per-core one-hot lhs, host-built in fp8: [ngroups, P, NCTOT, gw]."""
    bc = x_core.shape[0]
    oh = np.zeros((NCTOT * P, bc), NP_F8)
    # rows 0..31: bias / dense x16 / dense residual
    x16 = x_core[:, :N_DENSE].astype(np.float16)
    x8 = x16.astype(NP_F8).astype(np.float32)
    oh[0] = np.float32(1.0)
    oh[1:1 + N_DENSE] = x16.T.astype(NP_F8)
    oh[14:14 + N_DENSE] = (
        ((x16.astype(np.float32) - x8) * RES).astype(np.float16).T.astype(NP_F8))
    # sparse one-hot rows
    idx = x_core[:, N_DENSE:].astype(np.int64)  # [bc, 26]
    rows = SP0 + np.arange(N_SPARSE)[None, :] * SPARSE_DIM + idx  # [bc, 26]
    cols = np.broadcast_to(np.arange(bc)[:, None], rows.shape)
    oh[rows.ravel(), cols.ravel()] = np.float32(1.0)
    ngroups = bc // (GB * P)
    gw = GB * P
    return np.ascontiguousarray(
        oh.reshape(NCTOT, P, ngroups, gw).transpose(2, 1, 0, 3))


def run(x, w_weight, w_bias, v, trace=False, trace_kwargs=None):
    x = np.asarray(x, np.float32)
    w_weight = np.asarray(w_weight, np.float32)
    w_bias = np.asarray(w_bias, np.float32)
    v = np.asarray(v, np.float32)
    assert x.shape == (B_FULL, 39), x.shape

    vp8 = _prep_shared(w_weight, w_bias, v)
    in_maps = []
    for i in range(N_CORES):
        xc = x[i * BC:(i + 1) * BC]
        in_maps.append({
            "oh": _prep_core(xc),
            "vp": vp8,
        })

    nc = _get_program(BC)
    res = run_bass_kernel_spmd(
        nc, in_maps, list(range(N_CORES)),
        trace=trace, **(trace_kwargs or {}),
    )
    y = np.concatenate(
        [res.results[i]["y"].T.reshape(-1, 1) for i in range(N_CORES)], axis=0
    )
    return y.astype(np.float32), res


def kernel(x, w_weight, w_bias, v):
    y, _ = run(x, w_weight, w_bias, v)
    return y


# revision 16
# speedup vs baseline: 1.0487x; 1.0487x over previous
"""FFM layer kernel for 8 Trainium2 NeuronCores (fp8 DoubleRow edition, v3).

Math (reference): x[B,39] = 13 dense cols + 26 sparse index cols (ints 0..99
stored as f32).  inputs[B,2613] = [dense | one_hot(sparse)], then
  linear = inputs @ w.T + b
  field  = einsum('bn,nfk->bfk', inputs, v)        # [B,39,16]
  cross  = 0.5*sum_k((sum_f field)^2 - sum_f field^2)
  out    = sigmoid(linear + cross)

Strategy: data-parallel over batch, 2048 rows/core.  The one-hot matrix is
built host-side directly in fp8 (it is exactly the same number of bytes as
the index tensor an on-device build would need, so DMA cost is unchanged and
the DVE/GPSIMD build cost disappears entirely) and used as the stationary
operand of fp8 DoubleRow matmuls (256-row contraction per instruction, 2x
the fp16 PE rate):
  psum[128b, 658] = sum_pairs ohp[128f,2,128b].T @ vp[128f,2,658]
Columns: 624 field cols (k-major), then 16+16 split-precision s columns
(s_hi = fp8(A*512), s_res = fp8(residual*16)/16 with A = sum_f v — two fp8
levels give s = sum_f field to ~0.4% of |A|, so no row-reduce over fields is
needed), then w_hi/w_res (same trick for the linear term).  Dense x
precision is recovered by 13 residual feature rows carrying 16*(x16-x8)
against v/16, reusing rows that were zero padding.  Feature rows: [1s row
(bias) | 13 dense x | 13 dense residual | pad to 32 | 26*100 one-hot | zero
tail], packed into 22 chunks of 128 = 11 DoubleRow pairs (chunk 21 zero).
v scaled by 2048 into e4m3's normal range, fp32 PSUM accumulation.

Epilogue: ACT does the two Square-accums (sum field^2) and a per-group
batched sigmoid (logit pre-combined so the bias is 0); DVE does the s/w
column-pair reduces and the [p,4] logit combines.  Throwaway warmup matmuls
release the HAM clock throttle during the DMA head; host tensors are
partition-major and group-blocked so DMAs move long contiguous runs and the
first group's one-hot lands early.
"""

import sys

sys.path.insert(0, "/opt/trn_rl_repo")

import numpy as np
import ml_dtypes

import concourse.tile as tile
from concourse import bacc, mybir
from concourse.bass_utils import run_bass_kernel_spmd

N_CORES = 8
B_FULL = 16384
BC = B_FULL // N_CORES  # 2048 rows per core
P = 128
N_DENSE = 13
N_SPARSE = 26
SPARSE_DIM = 100
N_FIELD = 39
K_DIM = 16
NCHUNK = 21             # chunks carrying real feature rows
NCTOT = 22              # +1 zero chunk so DoubleRow pairs cover everything
NPAIR = NCTOT // 2
RTOT = NCHUNK * P       # 2688 real feature rows
# device feature rows: 0 = const-ones (bias), 1..13 = dense x, 14..26 =
# dense residual 16*(x16-x8), 27..31 = zero, then 26*100 one-hot, zero tail
SP0 = 32                # first one-hot row
NFEAT_END = SP0 + N_SPARSE * SPARSE_DIM  # 2632
NFCOL = N_FIELD * K_DIM  # 624 field cols
COLS0 = 512             # ps0: field cols 0:512
COLS = NFCOL + 2 * K_DIM + 2  # 658
COLS1 = COLS - COLS0    # 146: field 112 + s_hi 16 + s_res 16 + whi + wres
SH0 = NFCOL - COLS0     # 112 (ps1-local start of s_hi)
WH0 = SH0 + 2 * K_DIM   # 144 (ps1-local start of w cols)
GB = 4                  # batch tiles per group

SCALE_V = 2048.0
SCALE_W = 8192.0
SCALE_S = 512.0
RES = 16.0              # dense residual row scale-up

F8 = mybir.dt.float8e4
F32 = mybir.dt.float32
F16 = mybir.dt.float16
NP_F8 = ml_dtypes.float8_e4m3

_prog_cache = {}


def _build_program(bc):
    """One SPMD program for a batch slice of `bc` rows (all cores identical)."""
    nbt = bc // P
    ngroups = nbt // GB
    assert nbt % GB == 0
    gw = GB * P

    nc = bacc.Bacc("TRN2", target_bir_lowering=False, debug=False)
    # oh is group-blocked so each group's slice lands in one burst of long
    # contiguous per-partition runs; vp is partition-major for the same
    # reason
    oh_d = nc.declare_dram_parameter(
        "oh", [ngroups, P, NCTOT, gw], F8, isOutput=False)
    vp_d = nc.declare_dram_parameter("vp", [P, NCTOT, COLS], F8, isOutput=False)
    y_d = nc.declare_dram_parameter("y", [P, nbt], F32, isOutput=True)

    # per-group oh sub-DMAs over chunk ranges, split across the sync,
    # gpsimd and vector HWDGE queues (vp owns the scalar queue so it is
    # never stuck behind a one-hot burst — it gates every matmul); small
    # first sub so pair 0 lands early and the first matmuls start
    OSUB = [(0, 2), (2, 8), (8, 14), (14, NCTOT)]
    OSUB_ENG = ("sync", "gpsimd", "sync", "gpsimd")
    VSUB = [(0, 2), (2, 12), (12, NCTOT)]
    VSUB_ENG = ("scalar", "scalar", "scalar")

    with tile.TileContext(nc) as tc:
        with (
            tc.tile_pool(name="pers", bufs=1) as pers,
            tc.tile_pool(name="psum", bufs=3, space="PSUM") as psum,
            tc.tile_pool(name="epi", bufs=3) as epi,
            tc.tile_pool(name="grp", bufs=2) as grp,
        ):
            # group-major one-hot buffer: a group's DMA writes one fully
            # contiguous run per partition (22*512 B) instead of 512 B
            # strided bursts
            oh_all = pers.tile([P, ngroups, NCTOT, gw], F8, tag="ohall")
            y_all = pers.tile([P, nbt], F32, tag="yall")
            vp_all = pers.tile([P, NCTOT, COLS], F8, tag="vp")

            def load_oh(g):
                for (lo, hi), ename in zip(OSUB, OSUB_ENG):
                    getattr(nc, ename).dma_start(
                        oh_all[:, g, lo:hi, :], oh_d[g, :, lo:hi, :])

            def load_vp(lo, hi, ename):
                getattr(nc, ename).dma_start(
                    vp_all[:, lo:hi, :], vp_d[:, lo:hi, :])

            # warmup scratch first so the memsets aren't stuck behind DMA
            # issue on the gpsimd queue
            wz16 = pers.tile([P, 16], F16, tag="wz16")
            wz512 = pers.tile([P, 512], F16, tag="wz512")
            nc.gpsimd.memset(wz16[:], 0.0)
            nc.gpsimd.memset(wz512[:], 0.0)

            # vp chunks 0-1 first: they gate the first matmuls
            load_vp(*VSUB[0], VSUB_ENG[0])
            load_oh(0)
            for (lo, hi), ename in list(zip(VSUB, VSUB_ENG))[1:]:
                load_vp(lo, hi, ename)

            # PE warmup: throwaway matmuls on zeroed tiles during the DMA
            # head release the HAM clock throttle (cold PE runs slow until
            # ~3.4us of sustained activity); sized to end right when the
            # first one-hot pair and vp chunks have landed
            wps = psum.tile([P, 512], F32, tag="warmps", name="warmps", bufs=1)
            for _ in range(8):
                nc.tensor.matmul(wps[0:16, 0:512], wz16[:], wz512[:],
                                 start=True, stop=True)
            for _ in range(8):
                nc.tensor.matmul(wps[0:16, 0:64], wz16[:], wz512[:, 0:64],
                                 start=True, stop=True)

            for g in range(ngroups):
                if g + 1 < ngroups:
                    load_oh(g + 1)
                # per-group accumulator strips: one [128, GB] f32 tile each
                sq0b = grp.tile([P, GB], F32, tag="sq0b")
                sq1b = grp.tile([P, GB], F32, tag="sq1b")
                s2b = grp.tile([P, GB], F32, tag="s2b")
                ub = grp.tile([P, GB], F32, tag="ub")
                for b4 in range(GB):
                    bt = g * GB + b4
                    w0 = b4 * P
                    ps0 = psum.tile([P, COLS0], F32, tag="ps0")
                    ps1 = psum.tile([P, COLS1], F32, tag="ps1")
                    for j in range(NPAIR):
                        lhs = oh_all[:, g, 2 * j:2 * j + 2, w0:w0 + P]
                        nc.tensor.matmul(
                            ps0[:], lhs, vp_all[:, 2 * j:2 * j + 2, 0:COLS0],
                            start=(j == 0), stop=(j == NPAIR - 1),
                            perf_mode=mybir.MatmulPerfMode.DoubleRow,
                        )
                        nc.tensor.matmul(
                            ps1[:], lhs, vp_all[:, 2 * j:2 * j + 2, COLS0:COLS],
                            start=(j == 0), stop=(j == NPAIR - 1),
                            perf_mode=mybir.MatmulPerfMode.DoubleRow,
                        )
                    # s = s_hi + s_res (sigma=512 units); u = lin*8192
                    s_t = epi.tile([P, K_DIM], F32, tag="s")
                    nc.vector.tensor_reduce(
                        out=s_t[:],
                        in_=ps1[:, SH0:SH0 + 2 * K_DIM].rearrange(
                            "p (two k) -> p k two", two=2),
                        axis=mybir.AxisListType.X,
                        op=mybir.AluOpType.add,
                    )
                    nc.vector.tensor_reduce(
                        out=ub[:, b4:b4 + 1], in_=ps1[:, WH0:WH0 + 2],
                        axis=mybir.AxisListType.X,
                        op=mybir.AluOpType.add,
                    )
                    # sum field^2 on ACT: 512 cols from ps0, 112 from ps1
                    sq_scr = epi.tile([P, COLS0], F32, tag="sqscr")
                    nc.scalar.activation(
                        out=sq_scr[:], in_=ps0[:],
                        func=mybir.ActivationFunctionType.Square,
                        scale=1.0 / SCALE_V,
                        accum_out=sq0b[:, b4:b4 + 1],
                    )
                    sq_scr1 = epi.tile([P, SH0], F32, tag="sqscr1")
                    nc.scalar.activation(
                        out=sq_scr1[:], in_=ps1[:, 0:SH0],
                        func=mybir.ActivationFunctionType.Square,
                        scale=1.0 / SCALE_V,
                        accum_out=sq1b[:, b4:b4 + 1],
                    )
                    s2_scr = epi.tile([P, K_DIM], F32, tag="s2scr")
                    nc.scalar.activation(
                        out=s2_scr[:], in_=s_t[:],
                        func=mybir.ActivationFunctionType.Square,
                        scale=1.0 / SCALE_S,
                        accum_out=s2b[:, b4:b4 + 1],
                    )
                # batched logit combine + sigmoid for the group's 4 tiles:
                # L2 = (s2 - sq) + lin*2, y = Sigmoid(0.5*L2)
                sqs = grp.tile([P, GB], F32, tag="sqs")
                nc.vector.scalar_tensor_tensor(
                    out=sqs[:], in0=sq0b[:], scalar=1.0, in1=sq1b[:],
                    op0=mybir.AluOpType.mult, op1=mybir.AluOpType.add)
                dl = grp.tile([P, GB], F32, tag="dl")
                nc.vector.scalar_tensor_tensor(
                    out=dl[:], in0=sqs[:], scalar=-1.0, in1=s2b[:],
                    op0=mybir.AluOpType.mult, op1=mybir.AluOpType.add)
                l2 = grp.tile([P, GB], F32, tag="l2")
                nc.vector.scalar_tensor_tensor(
                    out=l2[:], in0=ub[:], scalar=2.0 / SCALE_W, in1=dl[:],
                    op0=mybir.AluOpType.mult, op1=mybir.AluOpType.add)
                nc.scalar.activation(
                    out=y_all[:, g * GB:(g + 1) * GB], in_=l2[:],
                    func=mybir.ActivationFunctionType.Sigmoid,
                    scale=0.5,
                )
                nc.scalar.dma_start(y_d[:, g * GB:(g + 1) * GB],
                                    y_all[:, g * GB:(g + 1) * GB])

    nc.compile()
    return nc


def _get_program(bc):
    if bc not in _prog_cache:
        _prog_cache[bc] = _build_program(bc)
    return _prog_cache[bc]


def _q8(a):
    return np.asarray(a, np.float32).astype(NP_F8).astype(np.float32)


def _prep_shared(w_weight, w_bias, v):
    """vp[P, 22, 658] fp8 (same on every core)."""
    w = w_weight[0].astype(np.float64)
    v_km = np.ascontiguousarray(
        v.astype(np.float64).transpose(0, 2, 1)).reshape(2613, NFCOL)
    A = v.astype(np.float64).sum(axis=1)  # [2613, 16]

    nrows = NCTOT * P
    vp = np.zeros((nrows, COLS), np.float32)

    def fill(rows, vblk, ablk, wblk):
        """Two-level fp8 quantization of a (v, A, w) block into device rows."""
        vp[rows, 0:NFCOL] = _q8(vblk * SCALE_V)
        s_hi = _q8(ablk * SCALE_S)
        vp[rows, NFCOL:NFCOL + K_DIM] = s_hi
        vp[rows, NFCOL + K_DIM:NFCOL + 2 * K_DIM] = _q8(
            (ablk - s_hi / SCALE_S) * SCALE_S * 16.0) / 16.0
        w_hi = _q8(wblk * SCALE_W)
        vp[rows, COLS - 2] = w_hi
        vp[rows, COLS - 1] = _q8((wblk - w_hi / SCALE_W) * SCALE_W * 16.0) / 16.0

    dn = np.arange(1, 1 + N_DENSE)
    fill(dn, v_km[:N_DENSE], A[:N_DENSE], w[:N_DENSE])
    dr = np.arange(14, 14 + N_DENSE)
    fill(dr, v_km[:N_DENSE] / RES, A[:N_DENSE] / RES, w[:N_DENSE] / RES)
    sp = np.arange(SP0, NFEAT_END)
    fill(sp, v_km[N_DENSE:], A[N_DENSE:], w[N_DENSE:])
    # bias via the const-ones row's w columns
    wb = float(w_bias[0])
    wb_hi = _q8(wb * SCALE_W)
    vp[0, COLS - 2] = wb_hi
    vp[0, COLS - 1] = _q8((wb - wb_hi / SCALE_W) * SCALE_W * 16.0) / 16.0

    vp8 = np.ascontiguousarray(
        vp.astype(NP_F8).reshape(NCTOT, P, COLS).transpose(1, 0, 2))
    return vp8


def _prep_core(x_core):
    """Per-core one-hot lhs, host-built in fp8: [ngroups, P, NCTOT, gw]."""
    bc = x_core.shape[0]
    oh = np.zeros((NCTOT * P, bc), NP_F8)
    # rows 0..31: bias / dense x16 / dense residual
    x16 = x_core[:, :N_DENSE].astype(np.float16)
    x8 = x16.astype(NP_F8).astype(np.float32)
    oh[0] = np.float32(1.0)
    oh[1:1 + N_DENSE] = x16.T.astype(NP_F8)
    oh[14:14 + N_DENSE] = (
        ((x16.astype(np.float32) - x8) * RES).astype(np.float16).T.astype(NP_F8))
    # sparse one-hot rows
    idx = x_core[:, N_DENSE:].astype(np.int64)  # [bc, 26]
    rows = SP0 + np.arange(N_SPARSE)[None, :] * SPARSE_DIM + idx  # [bc, 26]
    cols = np.broadcast_to(np.arange(bc)[:, None], rows.shape)
    oh[rows.ravel(), cols.ravel()] = np.float32(1.0)
    ngroups = bc // (GB * P)
    gw = GB * P
    return np.ascontiguousarray(
        oh.reshape(NCTOT, P, ngroups, gw).transpose(2, 1, 0, 3))


def run(x, w_weight, w_bias, v, trace=False, trace_kwargs=None):
    x = np.asarray(x, np.float32)
    w_weight = np.asarray(w_weight, np.float32)
    w_bias = np.asarray(w_bias, np.float32)
    v = np.asarray(v, np.float32)
    assert x.shape == (B_FULL, 39), x.shape

    vp8 = _prep_shared(w_weight, w_bias, v)
    in_maps = []
    for i in range(N_CORES):
        xc = x[i * BC:(i + 1) * BC]
        in_maps.append({
            "oh": _prep_core(xc),
            "vp": vp8,
        })

    nc = _get_program(BC)
    res = run_bass_kernel_spmd(
        nc, in_maps, list(range(N_CORES)),
        trace=trace, **(trace_kwargs or {}),
    )
    y = np.concatenate(
        [res.results[i]["y"].T.reshape(-1, 1) for i in range(N_CORES)], axis=0
    )
    return y.astype(np.float32), res


def kernel(x, w_weight, w_bias, v):
    y, _ = run(x, w_weight, w_bias, v)
    return y
